# revision 13
# baseline (speedup 1.0000x reference)
"""Trainium2 Bass kernel for nn_CrossAttention (B=4, N=2048, D=1024, 16 heads x 64).

Sharding: 8 cores = 4 batches x 2 head-groups (8 heads each).
Each core computes, for its (batch b, head-group g):
    q = x[b] @ Wq[g].T, k/v = ctx[b] @ Wk/Wv[g].T   (E = 512 inner dims)
    per head: ST[j,i] = (k_h q_h^T), P = exp(ST * scale) (no max-sub; logits small)
    OT_aug = [v_h | 1]^T-accumulated matmul -> rows 0..63 = unnormalized O^T,
      row 64 = softmax denominator L; normalize via reciprocal + selector matmul
    out_partial = O @ Wo[:, g].T  -> host sums the two partials per batch (+bo)

v2 schedule: the softmax exp stream on the Scalar(ACT) engine is the hard
floor (~33.5M elements/core at 1 elem/cycle/lane). The kernel is organized
so ACT starts within ~15us and never stalls:
  - head-pair-major global order: for hp: for ib: for jt -> ST pair (two
    concurrent K=64 row-group matmuls), exp, OT trailing one iteration
    (avoids head-of-line blocking of the PE queue on the exp result).
  - all projection work (kT/v/q/out-proj/norm) is emitted as small "filler"
    units interleaved into the attention stream, ordered by deadline:
    only kT[et0]+v[hp0] and q(ib0) are needed before the first exp.
  - bf16 datapath (PSUM stays f32): enables fast-weight-load, halves DMA
    and doubles DVE copy throughput. Normalization ops use f32r (fp32 slow
    path on the PE would cost 4 cyc/row).
"""

import os
from collections import deque

import numpy as np
import ml_dtypes

import concourse.bacc as bacc
import concourse.mybir as mybir
import concourse.tile as tile
from concourse.bass_utils import run_bass_kernel_spmd

f32 = mybir.dt.float32
f32r = mybir.dt.float32r
bf16 = mybir.dt.bfloat16

# Problem constants (hardcoded per contract)
B, NQ, NK = 4, 2048, 2048
D = 1024          # model dim
H, DH = 16, 64    # heads, head dim
G = 2             # head-groups (cores per batch)
HC = H // G       # heads per core = 8
E = HC * DH       # per-core inner dim = 512
P = 128
DO = D // P       # 8 contraction tiles for projections
EO = E // P       # 4
NJT = NK // P     # 16 j-tiles
NIB = 4           # i-blocks
IBW = NQ // NIB   # 512
NHP = HC // 2     # head pairs = 4
SCALE = DH ** -0.5

MM_DTYPE = os.environ.get("CROSSATT_MM_DTYPE", "bf16")
BDT = bf16 if MM_DTYPE == "bf16" else mybir.dt.float32r
NPDT = ml_dtypes.bfloat16 if MM_DTYPE == "bf16" else np.float32

# filler units consumed per attention iteration, front-loaded while the
# projection backlog races the first head-pair's exp stream
EARLY_ITERS = int(os.environ.get("CROSSATT_EARLY_ITERS", "80"))
EARLY_RATE = 2


def build_nc():
    nc = bacc.Bacc(None)
    # p-major host-tiled layouts: each partition's slice of any one DMA is
    # contiguous in DRAM (8KB segments instead of 8x1KB -> far fewer
    # descriptors, which is what bounds the prologue latency)
    xT = nc.declare_dram_parameter("xT", [P, NIB, DO, IBW], BDT, isOutput=False)
    cT = nc.declare_dram_parameter("cT", [P, 4, DO, 512], BDT, isOutput=False)
    wqT = nc.declare_dram_parameter("wqT", [P, DO, E], BDT, isOutput=False)
    wkT = nc.declare_dram_parameter("wkT", [P, DO, E], BDT, isOutput=False)
    wvT = nc.declare_dram_parameter("wvT", [P, DO, E], BDT, isOutput=False)
    woT = nc.declare_dram_parameter("woT", [P, EO, D], BDT, isOutput=False)
    outp = nc.declare_dram_parameter("outp", [NQ, D], f32, isOutput=True)

    xT_r = xT.ap()
    cT_r = cT.ap()
    wqT_r = wqT.ap()
    wkT_r = wkT.ap()
    wvT_r = wvT.ap()
    woT_r = woT.ap()
    out_ap = outp.ap()

    with tile.TileContext(nc) as tc:
        with (
            tc.tile_pool(name="persist", bufs=1) as persist,
            tc.tile_pool(name="weights", bufs=4) as wpool,
            tc.tile_pool(name="cblk", bufs=4) as cpool,
            tc.tile_pool(name="xblk", bufs=2) as xpool,
            tc.tile_pool(name="qpool", bufs=4) as qpool,
            tc.tile_pool(name="opool", bufs=4) as opool,
            tc.tile_pool(name="lrows", bufs=4) as lpool,
            tc.tile_pool(name="ptpool", bufs=3) as ptpool,
            tc.tile_pool(name="rcrows", bufs=1) as rcpool,
            tc.tile_pool(name="stg", bufs=2) as stgp,
            tc.tile_pool(name="ps_st", bufs=2, space="PSUM") as ps_st,
            tc.tile_pool(name="ps_ot", bufs=2, space="PSUM") as ps_ot,
            tc.tile_pool(name="ps_aux", bufs=2, space="PSUM") as ps_aux,
        ):
            kT_sb = persist.tile([P, EO, NK], BDT, tag="kT")
            v_sb = persist.tile([P, NJT, HC * 65], BDT, tag="v")
            v_r = v_sb.rearrange("p n (h c) -> p n h c", c=65)
            ones_stage = persist.tile([P, NJT, HC], f32, tag="ones_stage")
            nc.vector.memset(ones_stage[:], 1.0)
            nc.vector.tensor_copy(v_r[:, :, :, 64], ones_stage[:])
            # sel97[32s, s*64:(s+1)*64] = 1: lhsT selector so a K=97 matmul
            # broadcasts the reciprocal row at partition 32s across 64 output
            # partitions (engine access bases are limited to 0/32/64/96).
            sel_sb = persist.tile([97, 4 * 64], bf16, tag="sel")
            nc.vector.memset(sel_sb[:], 0.0)
            sel_r = sel_sb.rearrange("p (s c) -> p s c", c=64)
            for s in range(4):
                nc.vector.memset(sel_r[32 * s:32 * s + 1, s, :], 1.0)

            # PE warmup: zero-data matmuls keep the HAM activity monitor
            # busy during the input-DMA wait so the real projections start
            # at 2.4GHz instead of 1.2GHz
            zt = persist.tile([P, 512], bf16, tag="zt")
            nc.vector.memset(zt[:], 0.0)
            wps = ps_aux.tile([P, IBW], f32, tag="aux", name="warm")
            for _ in range(36):
                nc.tensor.matmul(wps[:], zt[:, 0:P], zt[:],
                                 start=True, stop=True)

            # per-i-block accumulators, live across the whole hp-major sweep
            oT = [opool.tile([P, EO, IBW], BDT, tag="o", name=f"oT{i}")
                  for i in range(NIB)]
            lrows = [lpool.tile([97, 2, IBW], f32, tag="lr", name=f"lrows{i}")
                     for i in range(NIB)]
            for t in lrows:
                nc.vector.memset(t[:], 1.0)

            # ---- input DMAs: only what the first exp needs goes first;
            # the rest is staggered so it doesn't steal HBM bandwidth ----
            wq_sb = wpool.tile([P, DO, E], BDT, tag="w")
            nc.sync.dma_start(wq_sb[:], wqT_r[:])
            xbs = [None] * NIB
            xbs[0] = xpool.tile([P, DO, 512], BDT, tag="x", name="xb")
            nc.sync.dma_start(xbs[0][:], xT_r[:, 0, :, :])
            wk_sb = wpool.tile([P, DO, E], BDT, tag="w")
            nc.sync.dma_start(wk_sb[:], wkT_r[:])
            wv_sb = wpool.tile([P, DO, E], BDT, tag="w")
            cblk = []
            for jb in range(4):
                cb = cpool.tile([P, DO, 512], BDT, tag="c", name=f"cb{jb}")
                cblk.append(cb)
            nc.sync.dma_start(cblk[0][:], cT_r[:, 0, :, :])
            wo_sb = wpool.tile([P, EO, D], BDT, tag="w")

            qT = [None] * NIB

            def aux_tile():
                return ps_aux.tile([P, IBW], f32, tag="aux", name="aux")

            # emission-order tracking: a consumer emitted before its
            # producer would get no dependency edge at all, so the main
            # loop force-drains filler until these flags flip
            kT_done = {}     # (jb, et) -> True once the kT copy is emitted
            v_done = {}      # (jt, hp) -> True once that j-tile's v copy is emitted
            qT_done = {}     # (ib, et) -> True once that q chunk copy is emitted

            # ---- filler generators: yield after each ~2-matmul unit ----
            def gen_kT_et(jb, et):
                ps = aux_tile()
                for do in range(DO):
                    nc.tensor.matmul(
                        ps[:], wk_sb[:, do, et * P:(et + 1) * P],
                        cblk[jb][:, do, :],
                        start=(do == 0), stop=(do == DO - 1))
                    if do % 2 == 1:
                        if do == DO - 1:
                            nc.vector.tensor_copy(
                                kT_sb[:, et, jb * 512:(jb + 1) * 512], ps[:])
                            kT_done[(jb, et)] = True
                        yield

            def gen_v(jb, hp):
                # per j-tile: [128j, 128e] for heads (2hp, 2hp+1)
                for j2 in range(4):
                    jt = jb * 4 + j2
                    ps = aux_tile()
                    for do in range(DO):
                        nc.tensor.matmul(
                            ps[:, 0:P], cblk[jb][:, do, j2 * P:(j2 + 1) * P],
                            wv_sb[:, do, hp * P:(hp + 1) * P],
                            start=(do == 0), stop=(do == DO - 1))
                        if do == 3:
                            yield
                    nc.vector.tensor_copy(
                        v_r[:, jt, 2 * hp:2 * hp + 2, 0:64],
                        ps[:, 0:P].rearrange("p (h c) -> p h c", c=64))
                    v_done[(jt, hp)] = True
                    yield

            def gen_vrest(jb):
                # heads 2..7 in one N=384 sweep per j-tile
                for j2 in range(4):
                    jt = jb * 4 + j2
                    ps = aux_tile()
                    for do in range(DO):
                        nc.tensor.matmul(
                            ps[:, 0:384], cblk[jb][:, do, j2 * P:(j2 + 1) * P],
                            wv_sb[:, do, P:E],
                            start=(do == 0), stop=(do == DO - 1))
                        if do == 3:
                            yield
                    nc.vector.tensor_copy(
                        v_r[:, jt, 2:HC, 0:64],
                        ps[:, 0:384].rearrange("p (h c) -> p h c", c=64))
                    for hp_ in range(1, NHP):
                        v_done[(jt, hp_)] = True
                    yield

            def gen_cdma(jb):
                nc.sync.dma_start(cblk[jb][:], cT_r[:, jb, :, :])
                yield

            def gen_wodma():
                nc.sync.dma_start(wo_sb[:], woT_r[:])
                yield

            def gen_xdma(ib):
                xb = xpool.tile([P, DO, 512], BDT, tag="x", name="xb")
                nc.sync.dma_start(xb[:], xT_r[:, ib, :, :])
                xbs[ib] = xb
                yield

            def gen_qproj(ib):
                qt = qpool.tile([P, EO, IBW], BDT, tag="q", name="qt")
                qT[ib] = qt
                xb = xbs[ib]
                for et in range(EO):
                    ps = aux_tile()
                    for do in range(DO):
                        nc.tensor.matmul(
                            ps[:], wq_sb[:, do, et * P:(et + 1) * P],
                            xb[:, do, :],
                            start=(do == 0), stop=(do == DO - 1))
                        if do % 2 == 1:
                            if do == DO - 1:
                                nc.vector.tensor_copy(qt[:, et, :], ps[:])
                                qT_done[(ib, et)] = True
                            yield

            rcs = [None] * NIB

            def gen_recip(ib):
                # separate generator, split into halves: the DVE reciprocal
                # must be done before gen_normout's selector matmuls enter
                # the PE queue, else they head-of-line block the score
                # stream (and the resulting PE idle re-throttles HAM)
                rc = rcpool.tile([97, 2, IBW], bf16, tag="rc", name="rc")
                rcs[ib] = rc
                with nc.allow_low_precision(reason="softmax denom in bf16"):
                    nc.vector.reciprocal(rc[:, 0, :], lrows[ib][:, 0, :])
                yield
                with nc.allow_low_precision(reason="softmax denom in bf16"):
                    nc.vector.reciprocal(rc[:, 1, :], lrows[ib][:, 1, :])
                yield

            def gen_normout(ib):
                # empty units sized to the first reciprocal half; ho-major
                # order below means slot-1 selectors only run after slot-0
                # ones, by which time the second half is done too
                for _ in range(10):
                    yield
                rc = rcs[ib]
                for ho in range(2):
                    base = 64 * ho
                    for hp in range(NHP):
                        bc = aux_tile()
                        nc.tensor.matmul(
                            bc[0:64, :], sel_sb[:, hp * 64:(hp + 1) * 64],
                            rc[:, ho, :], start=True, stop=True)
                        nc.vector.tensor_mul(
                            out=oT[ib][base:base + 64, hp, :],
                            in0=oT[ib][base:base + 64, hp, :],
                            in1=bc[0:64, :])
                        if hp % 2 == 1:
                            yield
                for it in range(4):
                    for ob in range(2):
                        ps = aux_tile()
                        for eo in range(EO):
                            nc.tensor.matmul(
                                ps[:],
                                oT[ib][:, eo, it * P:(it + 1) * P],
                                wo_sb[:, eo, ob * IBW:(ob + 1) * IBW],
                                start=(eo == 0), stop=(eo == EO - 1))
                            if eo % 2 == 1:
                                if eo == EO - 1:
                                    sg = stgp.tile([P, IBW], f32, tag="sg", name="sg")
                                    nc.vector.tensor_copy(sg[:], ps[:])
                                    r0 = ib * IBW + it * P
                                    nc.sync.dma_start(
                                        out_ap[r0:r0 + P,
                                               ob * IBW:(ob + 1) * IBW],
                                        sg[:])
                                yield

            # ---- prologue: everything the first exp needs, inline ----
            def run_all(g):
                for _ in g:
                    pass

            run_all(gen_qproj(0))
            run_all(gen_kT_et(0, 0))
            # deferred DMAs: fewer active streams while wq/x0/wk/c0 land
            nc.sync.dma_start(wv_sb[:], wvT_r[:])
            run_all(gen_cdma(1))
            run_all(gen_cdma(2))
            run_all(gen_cdma(3))

            filler = deque()

            def consume(n):
                done = 0
                while done < n and filler:
                    try:
                        next(filler[0])
                        done += 1
                    except StopIteration:
                        filler.popleft()

            filler.extend([
                gen_v(0, 0),
                gen_kT_et(1, 0),
                gen_v(1, 0),
                gen_kT_et(2, 0),
                gen_v(2, 0),
                gen_kT_et(3, 0),
                gen_v(3, 0),
                gen_xdma(1), gen_qproj(1),
                gen_wodma(),
                gen_xdma(2), gen_qproj(2),
                gen_xdma(3), gen_qproj(3),
                gen_kT_et(0, 1), gen_kT_et(1, 1),
                gen_kT_et(2, 1), gen_kT_et(3, 1),
                gen_vrest(0), gen_vrest(1), gen_vrest(2), gen_vrest(3),
                gen_kT_et(0, 2), gen_kT_et(1, 2),
                gen_kT_et(2, 2), gen_kT_et(3, 2),
                gen_kT_et(0, 3), gen_kT_et(1, 3),
                gen_kT_et(2, 3), gen_kT_et(3, 3),
            ])

            # ---- attention sweep: hp-major, OT trailing one iteration ----
            state = {"pending": None, "ot_pair": None}

            def flush_pending():
                if state["pending"] is None:
                    return
                php, pib, pjt, ppt = state["pending"]
                while not v_done.get((pjt, php)):
                    consume(1)
                if pjt == 0:
                    state["ot_pair"] = (
                        ps_ot.tile([P, IBW], f32, tag="ot", name="otA"),
                        ps_ot.tile([P, IBW], f32, tag="ot", name="otB"),
                    )
                for slot in (0, 1):
                    hl = 2 * php + slot
                    nc.tensor.matmul(
                        state["ot_pair"][slot][0:65, :],
                        v_sb[:, pjt, hl * 65:(hl + 1) * 65],
                        ppt[:, slot, :],
                        start=(pjt == 0), stop=(pjt == NJT - 1))
                if pjt == NJT - 1:
                    for ho, ot in enumerate(state["ot_pair"]):
                        nc.vector.tensor_copy(
                            oT[pib][64 * ho:64 * ho + 64, php, :],
                            ot[0:64, :])
                        nc.vector.tensor_copy(
                            lrows[pib][32 * php:32 * php + 1, ho, :],
                            ot[64:65, :])
                    state["ot_pair"] = None
                    if php == NHP - 1:
                        filler.append(gen_recip(pib))
                        filler.append(gen_normout(pib))
                state["pending"] = None

            g = 0
            for hp in range(NHP):
                for ib in range(NIB):
                    for jt in range(NJT):
                        while (not qT_done.get((ib, hp))
                               or not kT_done.get((jt // 4, hp))):
                            consume(1)
                        st = ps_st.tile([P, 2, IBW], f32, tag="st", name="st")
                        for slot in (0, 1):
                            lo = slot * 64
                            nc.tensor.matmul(
                                st[:, slot, :],
                                kT_sb[lo:lo + 64, hp, jt * P:(jt + 1) * P],
                                qT[ib][lo:lo + 64, hp, :],
                                start=True, stop=True)
                        pt = ptpool.tile([P, 2, IBW], BDT, tag="pt", name="pt")
                        nc.scalar.activation(
                            pt[:], st[:], mybir.ActivationFunctionType.Exp,
                            scale=SCALE)
                        if g < 16 or g >= 192:
                            rate = 3
                        elif g < EARLY_ITERS:
                            rate = EARLY_RATE
                        else:
                            rate = 1
                        consume(rate)
                        flush_pending()
                        state["pending"] = (hp, ib, jt, pt)
                        g += 1

            flush_pending()
            consume(1 << 30)

    nc.finalize()
    return nc


_NC_CACHE = None


def _get_nc():
    global _NC_CACHE
    if _NC_CACHE is None:
        _NC_CACHE = build_nc()
    return _NC_CACHE


def make_in_maps(x, context, Wq, Wk, Wv, Wo):
    in_maps = []
    for c in range(8):
        b, g = divmod(c, 2)
        es = slice(g * E, (g + 1) * E)
        def act_tile(a, nblk, w):
            # [N, D] -> [128, nblk, DO, w]: partition-major, contiguous per
            # (partition, block) 8KB DMA segment
            return np.ascontiguousarray(
                a.reshape(nblk, w, DO, P).transpose(3, 0, 2, 1)).astype(NPDT)

        def w_tile(wt, ko):
            # [D_in, N_out] -> [128, ko, N_out]
            n_out = wt.shape[1]
            return np.ascontiguousarray(
                wt.reshape(ko, P, n_out).transpose(1, 0, 2)).astype(NPDT)

        in_maps.append({
            "xT": act_tile(x[b], NIB, IBW),
            "cT": act_tile(context[b], 4, 512),
            "wqT": w_tile(Wq[es].T, DO),
            "wkT": w_tile(Wk[es].T, DO),
            "wvT": w_tile(Wv[es].T, DO),
            "woT": w_tile(Wo[:, es].T, EO),
        })
    return in_maps


def kernel(**inputs):
    x = np.asarray(inputs["x"], dtype=np.float32)
    context = np.asarray(inputs["context"], dtype=np.float32)
    Wq = np.asarray(inputs["Wq"], dtype=np.float32)
    Wk = np.asarray(inputs["Wk"], dtype=np.float32)
    Wv = np.asarray(inputs["Wv"], dtype=np.float32)
    Wo = np.asarray(inputs["Wo"], dtype=np.float32)
    bo = np.asarray(inputs["bo"], dtype=np.float32)

    nc = _get_nc()
    in_maps = make_in_maps(x, context, Wq, Wk, Wv, Wo)
    res = run_bass_kernel_spmd(nc, in_maps, list(range(8)))
    out = np.zeros((B, NQ, D), np.float32)
    for c in range(8):
        out[c // 2] += res.results[c]["outp"]
    out += bo[None, None, :]
    return out


if __name__ == "__main__":
    nc = build_nc()
    print("built ok; instructions:", len(nc.inst_map))


# revision 14
# speedup vs baseline: 1.0241x; 1.0241x over previous
"""Trainium2 Bass kernel for nn_CrossAttention (B=4, N=2048, D=1024, 16 heads x 64).

Sharding: 8 cores = 4 batches x 2 head-groups (8 heads each).
Each core computes, for its (batch b, head-group g):
    q = x[b] @ Wq[g].T, k/v = ctx[b] @ Wk/Wv[g].T   (E = 512 inner dims)
    per head: ST[j,i] = (k_h q_h^T), P = exp(ST * scale) (no max-sub; logits small)
    OT_aug = [v_h | 1]^T-accumulated matmul -> rows 0..63 = unnormalized O^T,
      row 64 = softmax denominator L; normalize via reciprocal + selector matmul
    out_partial = O @ Wo[:, g].T  -> host sums the two partials per batch (+bo)

v2 schedule: the softmax exp stream on the Scalar(ACT) engine is the hard
floor (~33.5M elements/core at 1 elem/cycle/lane). The kernel is organized
so ACT starts within ~15us and never stalls:
  - head-pair-major global order: for hp: for ib: for jt -> ST pair (two
    concurrent K=64 row-group matmuls), exp, OT trailing one iteration
    (avoids head-of-line blocking of the PE queue on the exp result).
  - all projection work (kT/v/q/out-proj/norm) is emitted as small "filler"
    units interleaved into the attention stream, ordered by deadline:
    only kT[et0]+v[hp0] and q(ib0) are needed before the first exp.
  - bf16 datapath (PSUM stays f32): enables fast-weight-load, halves DMA
    and doubles DVE copy throughput. Normalization ops use f32r (fp32 slow
    path on the PE would cost 4 cyc/row).
"""

import os
from collections import deque

import numpy as np
import ml_dtypes

import concourse.bacc as bacc
import concourse.mybir as mybir
import concourse.tile as tile
from concourse.bass_utils import run_bass_kernel_spmd

f32 = mybir.dt.float32
f32r = mybir.dt.float32r
bf16 = mybir.dt.bfloat16

# Problem constants (hardcoded per contract)
B, NQ, NK = 4, 2048, 2048
D = 1024          # model dim
H, DH = 16, 64    # heads, head dim
G = 2             # head-groups (cores per batch)
HC = H // G       # heads per core = 8
E = HC * DH       # per-core inner dim = 512
P = 128
DO = D // P       # 8 contraction tiles for projections
EO = E // P       # 4
NJT = NK // P     # 16 j-tiles
NIB = 4           # i-blocks
IBW = NQ // NIB   # 512
NHP = HC // 2     # head pairs = 4
SCALE = DH ** -0.5

MM_DTYPE = os.environ.get("CROSSATT_MM_DTYPE", "bf16")
BDT = bf16 if MM_DTYPE == "bf16" else mybir.dt.float32r
NPDT = ml_dtypes.bfloat16 if MM_DTYPE == "bf16" else np.float32

# filler units consumed per attention iteration, front-loaded while the
# projection backlog races the first head-pair's exp stream
EARLY_ITERS = int(os.environ.get("CROSSATT_EARLY_ITERS", "80"))
EARLY_RATE = 2


def build_nc():
    nc = bacc.Bacc(None)
    # p-major host-tiled layouts: each partition's slice of any one DMA is
    # contiguous in DRAM (8KB segments instead of 8x1KB -> far fewer
    # descriptors, which is what bounds the prologue latency)
    xT = nc.declare_dram_parameter("xT", [P, NIB, DO, IBW], BDT, isOutput=False)
    cT = nc.declare_dram_parameter("cT", [P, 4, DO, 512], BDT, isOutput=False)
    wqT = nc.declare_dram_parameter("wqT", [P, DO, E], BDT, isOutput=False)
    wkT = nc.declare_dram_parameter("wkT", [P, DO, E], BDT, isOutput=False)
    wvT = nc.declare_dram_parameter("wvT", [P, DO, E], BDT, isOutput=False)
    woT = nc.declare_dram_parameter("woT", [P, EO, D], BDT, isOutput=False)
    outp = nc.declare_dram_parameter("outp", [NQ, D], f32, isOutput=True)

    xT_r = xT.ap()
    cT_r = cT.ap()
    wqT_r = wqT.ap()
    wkT_r = wkT.ap()
    wvT_r = wvT.ap()
    woT_r = woT.ap()
    out_ap = outp.ap()

    with tile.TileContext(nc) as tc:
        with (
            tc.tile_pool(name="persist", bufs=1) as persist,
            tc.tile_pool(name="weights", bufs=4) as wpool,
            tc.tile_pool(name="cblk", bufs=4) as cpool,
            tc.tile_pool(name="xblk", bufs=2) as xpool,
            tc.tile_pool(name="qpool", bufs=4) as qpool,
            tc.tile_pool(name="opool", bufs=4) as opool,
            tc.tile_pool(name="lrows", bufs=4) as lpool,
            tc.tile_pool(name="ptpool", bufs=3) as ptpool,
            tc.tile_pool(name="rcrows", bufs=1) as rcpool,
            tc.tile_pool(name="stg", bufs=2) as stgp,
            tc.tile_pool(name="ps_st", bufs=2, space="PSUM") as ps_st,
            tc.tile_pool(name="ps_ot", bufs=2, space="PSUM") as ps_ot,
            tc.tile_pool(name="ps_aux", bufs=2, space="PSUM") as ps_aux,
        ):
            kT_sb = persist.tile([P, EO, NK], BDT, tag="kT")
            v_sb = persist.tile([P, NJT, HC * 65], BDT, tag="v")
            v_r = v_sb.rearrange("p n (h c) -> p n h c", c=65)
            ones_stage = persist.tile([P, NJT, HC], f32, tag="ones_stage")
            nc.vector.memset(ones_stage[:], 1.0)
            nc.vector.tensor_copy(v_r[:, :, :, 64], ones_stage[:])
            # sel97[32s, s*64:(s+1)*64] = 1: lhsT selector so a K=97 matmul
            # broadcasts the reciprocal row at partition 32s across 64 output
            # partitions (engine access bases are limited to 0/32/64/96).
            sel_sb = persist.tile([97, 4 * 64], bf16, tag="sel")
            nc.vector.memset(sel_sb[:], 0.0)
            sel_r = sel_sb.rearrange("p (s c) -> p s c", c=64)
            for s in range(4):
                nc.vector.memset(sel_r[32 * s:32 * s + 1, s, :], 1.0)

            # PE warmup: zero-data matmuls keep the HAM activity monitor
            # busy during the input-DMA wait so the real projections start
            # at 2.4GHz instead of 1.2GHz
            zt = persist.tile([P, 512], bf16, tag="zt")
            nc.vector.memset(zt[:], 0.0)
            wps = ps_aux.tile([P, IBW], f32, tag="aux", name="warm")
            for _ in range(12):
                nc.tensor.matmul(wps[:], zt[:, 0:P], zt[:],
                                 start=True, stop=True)

            # per-i-block accumulators, live across the whole hp-major sweep
            oT = [opool.tile([P, EO, IBW], BDT, tag="o", name=f"oT{i}")
                  for i in range(NIB)]
            lrows = [lpool.tile([97, 2, IBW], f32, tag="lr", name=f"lrows{i}")
                     for i in range(NIB)]
            for t in lrows:
                nc.vector.memset(t[:], 1.0)

            # ---- input DMAs: only what the first exp needs goes first;
            # the rest is staggered so it doesn't steal HBM bandwidth ----
            wq_sb = wpool.tile([P, DO, E], BDT, tag="w")
            nc.sync.dma_start(wq_sb[:], wqT_r[:])
            xbs = [None] * NIB
            xbs[0] = xpool.tile([P, DO, 512], BDT, tag="x", name="xb")
            nc.sync.dma_start(xbs[0][:], xT_r[:, 0, :, :])
            wk_sb = wpool.tile([P, DO, E], BDT, tag="w")
            nc.sync.dma_start(wk_sb[:], wkT_r[:])
            wv_sb = wpool.tile([P, DO, E], BDT, tag="w")
            cblk = []
            for jb in range(4):
                cb = cpool.tile([P, DO, 512], BDT, tag="c", name=f"cb{jb}")
                cblk.append(cb)
            nc.sync.dma_start(cblk[0][:], cT_r[:, 0, :, :])
            wo_sb = wpool.tile([P, EO, D], BDT, tag="w")

            qT = [None] * NIB

            def aux_tile():
                return ps_aux.tile([P, IBW], f32, tag="aux", name="aux")

            # emission-order tracking: a consumer emitted before its
            # producer would get no dependency edge at all, so the main
            # loop force-drains filler until these flags flip
            kT_done = {}     # (jb, et) -> True once the kT copy is emitted
            v_done = {}      # (jt, hp) -> True once that j-tile's v copy is emitted
            qT_done = {}     # (ib, et) -> True once that q chunk copy is emitted

            # ---- filler generators: yield after each ~2-matmul unit ----
            def gen_kT_et(jb, et):
                ps = aux_tile()
                for do in range(DO):
                    nc.tensor.matmul(
                        ps[:], wk_sb[:, do, et * P:(et + 1) * P],
                        cblk[jb][:, do, :],
                        start=(do == 0), stop=(do == DO - 1))
                    if do % 2 == 1:
                        if do == DO - 1:
                            nc.vector.tensor_copy(
                                kT_sb[:, et, jb * 512:(jb + 1) * 512], ps[:])
                            kT_done[(jb, et)] = True
                        yield

            def gen_v(jb, hp):
                # per j-tile: [128j, 128e] for heads (2hp, 2hp+1)
                for j2 in range(4):
                    jt = jb * 4 + j2
                    ps = aux_tile()
                    for do in range(DO):
                        nc.tensor.matmul(
                            ps[:, 0:P], cblk[jb][:, do, j2 * P:(j2 + 1) * P],
                            wv_sb[:, do, hp * P:(hp + 1) * P],
                            start=(do == 0), stop=(do == DO - 1))
                        if do == 3:
                            yield
                    nc.vector.tensor_copy(
                        v_r[:, jt, 2 * hp:2 * hp + 2, 0:64],
                        ps[:, 0:P].rearrange("p (h c) -> p h c", c=64))
                    v_done[(jt, hp)] = True
                    yield

            def gen_vrest(jb):
                # heads 2..7 in one N=384 sweep per j-tile
                for j2 in range(4):
                    jt = jb * 4 + j2
                    ps = aux_tile()
                    for do in range(DO):
                        nc.tensor.matmul(
                            ps[:, 0:384], cblk[jb][:, do, j2 * P:(j2 + 1) * P],
                            wv_sb[:, do, P:E],
                            start=(do == 0), stop=(do == DO - 1))
                        if do == 3:
                            yield
                    nc.vector.tensor_copy(
                        v_r[:, jt, 2:HC, 0:64],
                        ps[:, 0:384].rearrange("p (h c) -> p h c", c=64))
                    for hp_ in range(1, NHP):
                        v_done[(jt, hp_)] = True
                    yield

            def gen_cdma(jb):
                nc.sync.dma_start(cblk[jb][:], cT_r[:, jb, :, :])
                yield

            def gen_wodma():
                nc.sync.dma_start(wo_sb[:], woT_r[:])
                yield

            def gen_xdma(ib):
                xb = xpool.tile([P, DO, 512], BDT, tag="x", name="xb")
                nc.sync.dma_start(xb[:], xT_r[:, ib, :, :])
                xbs[ib] = xb
                yield

            def gen_qproj(ib):
                qt = qpool.tile([P, EO, IBW], BDT, tag="q", name="qt")
                qT[ib] = qt
                xb = xbs[ib]
                for et in range(EO):
                    ps = aux_tile()
                    for do in range(DO):
                        nc.tensor.matmul(
                            ps[:], wq_sb[:, do, et * P:(et + 1) * P],
                            xb[:, do, :],
                            start=(do == 0), stop=(do == DO - 1))
                        if do % 2 == 1:
                            if do == DO - 1:
                                nc.vector.tensor_copy(qt[:, et, :], ps[:])
                                qT_done[(ib, et)] = True
                            yield

            rcs = [None] * NIB
            gstate = {"g": 0, "done": False}

            def emit_recip(ib):
                # emitted inline at the drain; split in halves so the first
                # selector matmuls only need the first half
                rc = rcpool.tile([97, 2, IBW], bf16, tag="rc", name="rc")
                rcs[ib] = rc
                with nc.allow_low_precision(reason="softmax denom in bf16"):
                    nc.vector.reciprocal(rc[:, 0, :], lrows[ib][:, 0, :])
                    nc.vector.reciprocal(rc[:, 1, :], lrows[ib][:, 1, :])

            def gen_normout(ib):
                # wait REAL iterations (not consume-units): the selector
                # matmuls head-of-line block the PE queue if they are
                # emitted before the DVE reciprocal has finished
                g0 = gstate["g"]
                while gstate["g"] < g0 + 8 and not gstate["done"]:
                    yield
                rc = rcs[ib]
                for ho in range(2):
                    if ho == 1:
                        while gstate["g"] < g0 + 11 and not gstate["done"]:
                            yield
                    base = 64 * ho
                    for hp in range(NHP):
                        bc = aux_tile()
                        nc.tensor.matmul(
                            bc[0:64, :], sel_sb[:, hp * 64:(hp + 1) * 64],
                            rc[:, ho, :], start=True, stop=True)
                        nc.vector.tensor_mul(
                            out=oT[ib][base:base + 64, hp, :],
                            in0=oT[ib][base:base + 64, hp, :],
                            in1=bc[0:64, :])
                        if hp % 2 == 1:
                            yield
                for it in range(4):
                    for ob in range(2):
                        ps = aux_tile()
                        for eo in range(EO):
                            nc.tensor.matmul(
                                ps[:],
                                oT[ib][:, eo, it * P:(it + 1) * P],
                                wo_sb[:, eo, ob * IBW:(ob + 1) * IBW],
                                start=(eo == 0), stop=(eo == EO - 1))
                            if eo % 2 == 1:
                                if eo == EO - 1:
                                    sg = stgp.tile([P, IBW], f32, tag="sg", name="sg")
                                    nc.vector.tensor_copy(sg[:], ps[:])
                                    r0 = ib * IBW + it * P
                                    nc.sync.dma_start(
                                        out_ap[r0:r0 + P,
                                               ob * IBW:(ob + 1) * IBW],
                                        sg[:])
                                yield

            # ---- prologue: everything the first exp needs, inline ----
            def run_all(g):
                for _ in g:
                    pass

            run_all(gen_qproj(0))
            run_all(gen_kT_et(0, 0))
            # deferred DMAs: fewer active streams while wq/x0/wk/c0 land
            nc.sync.dma_start(wv_sb[:], wvT_r[:])
            run_all(gen_cdma(1))
            run_all(gen_cdma(2))
            run_all(gen_cdma(3))

            filler = deque()

            def consume(n):
                done = 0
                while done < n and filler:
                    try:
                        next(filler[0])
                        done += 1
                    except StopIteration:
                        filler.popleft()

            filler.extend([
                gen_v(0, 0),
                gen_kT_et(1, 0),
                gen_v(1, 0),
                gen_kT_et(2, 0),
                gen_v(2, 0),
                gen_kT_et(3, 0),
                gen_v(3, 0),
                gen_xdma(1), gen_qproj(1),
                gen_wodma(),
                gen_xdma(2), gen_qproj(2),
                gen_xdma(3), gen_qproj(3),
                gen_kT_et(0, 1), gen_kT_et(1, 1),
                gen_kT_et(2, 1), gen_kT_et(3, 1),
                gen_vrest(0), gen_vrest(1), gen_vrest(2), gen_vrest(3),
                gen_kT_et(0, 2), gen_kT_et(1, 2),
                gen_kT_et(2, 2), gen_kT_et(3, 2),
                gen_kT_et(0, 3), gen_kT_et(1, 3),
                gen_kT_et(2, 3), gen_kT_et(3, 3),
            ])

            # ---- attention sweep: hp-major, OT trailing one iteration ----
            state = {"pending": None, "ot_pair": None}

            def flush_pending():
                if state["pending"] is None:
                    return
                php, pib, pjt, ppt = state["pending"]
                while not v_done.get((pjt, php)):
                    consume(1)
                if pjt == 0:
                    state["ot_pair"] = (
                        ps_ot.tile([P, IBW], f32, tag="ot", name="otA"),
                        ps_ot.tile([P, IBW], f32, tag="ot", name="otB"),
                    )
                for slot in (0, 1):
                    hl = 2 * php + slot
                    nc.tensor.matmul(
                        state["ot_pair"][slot][0:65, :],
                        v_sb[:, pjt, hl * 65:(hl + 1) * 65],
                        ppt[:, slot, :],
                        start=(pjt == 0), stop=(pjt == NJT - 1))
                if pjt == NJT - 1:
                    for ho, ot in enumerate(state["ot_pair"]):
                        nc.vector.tensor_copy(
                            oT[pib][64 * ho:64 * ho + 64, php, :],
                            ot[0:64, :])
                        nc.vector.tensor_copy(
                            lrows[pib][32 * php:32 * php + 1, ho, :],
                            ot[64:65, :])
                    state["ot_pair"] = None
                    if php == NHP - 1:
                        emit_recip(pib)
                        filler.append(gen_normout(pib))
                state["pending"] = None

            g = 0
            for hp in range(NHP):
                for ib in range(NIB):
                    for jt in range(NJT):
                        while (not qT_done.get((ib, hp))
                               or not kT_done.get((jt // 4, hp))):
                            consume(1)
                        st = ps_st.tile([P, 2, IBW], f32, tag="st", name="st")
                        for slot in (0, 1):
                            lo = slot * 64
                            nc.tensor.matmul(
                                st[:, slot, :],
                                kT_sb[lo:lo + 64, hp, jt * P:(jt + 1) * P],
                                qT[ib][lo:lo + 64, hp, :],
                                start=True, stop=True)
                        pt = ptpool.tile([P, 2, IBW], BDT, tag="pt", name="pt")
                        nc.scalar.activation(
                            pt[:], st[:], mybir.ActivationFunctionType.Exp,
                            scale=SCALE)
                        if g < 16 or g >= 192:
                            rate = 3
                        elif g < EARLY_ITERS:
                            rate = EARLY_RATE
                        else:
                            rate = 1
                        consume(rate)
                        flush_pending()
                        state["pending"] = (hp, ib, jt, pt)
                        g += 1
                        gstate["g"] = g

            gstate["done"] = True
            flush_pending()
            consume(1 << 30)

    nc.finalize()
    return nc


_NC_CACHE = None


def _get_nc():
    global _NC_CACHE
    if _NC_CACHE is None:
        _NC_CACHE = build_nc()
    return _NC_CACHE


def make_in_maps(x, context, Wq, Wk, Wv, Wo):
    in_maps = []
    for c in range(8):
        b, g = divmod(c, 2)
        es = slice(g * E, (g + 1) * E)
        def act_tile(a, nblk, w):
            # [N, D] -> [128, nblk, DO, w]: partition-major, contiguous per
            # (partition, block) 8KB DMA segment
            return np.ascontiguousarray(
                a.reshape(nblk, w, DO, P).transpose(3, 0, 2, 1)).astype(NPDT)

        def w_tile(wt, ko):
            # [D_in, N_out] -> [128, ko, N_out]
            n_out = wt.shape[1]
            return np.ascontiguousarray(
                wt.reshape(ko, P, n_out).transpose(1, 0, 2)).astype(NPDT)

        in_maps.append({
            "xT": act_tile(x[b], NIB, IBW),
            "cT": act_tile(context[b], 4, 512),
            "wqT": w_tile(Wq[es].T, DO),
            "wkT": w_tile(Wk[es].T, DO),
            "wvT": w_tile(Wv[es].T, DO),
            "woT": w_tile(Wo[:, es].T, EO),
        })
    return in_maps


def kernel(**inputs):
    x = np.asarray(inputs["x"], dtype=np.float32)
    context = np.asarray(inputs["context"], dtype=np.float32)
    Wq = np.asarray(inputs["Wq"], dtype=np.float32)
    Wk = np.asarray(inputs["Wk"], dtype=np.float32)
    Wv = np.asarray(inputs["Wv"], dtype=np.float32)
    Wo = np.asarray(inputs["Wo"], dtype=np.float32)
    bo = np.asarray(inputs["bo"], dtype=np.float32)

    nc = _get_nc()
    in_maps = make_in_maps(x, context, Wq, Wk, Wv, Wo)
    res = run_bass_kernel_spmd(nc, in_maps, list(range(8)))
    out = np.zeros((B, NQ, D), np.float32)
    for c in range(8):
        out[c // 2] += res.results[c]["outp"]
    out += bo[None, None, :]
    return out


if __name__ == "__main__":
    nc = build_nc()
    print("built ok; instructions:", len(nc.inst_map))
